# revision 8
# baseline (speedup 1.0000x reference)
"""Trainium2 Bass kernel for nn_MixtureOfAdaptors (moe_routing).

The reference routing collapses to expert 0 with weight 1.0, so the module is
exactly: out = x @ W[0].T + b[0], with x [65536, 1024] fp32.

Strategy (8 NeuronCores, data-parallel over tokens):
  - Host: shard x by token into 8 x [8192, 1024]; transpose each shard to
    feature-major [1024, 8192] (the PE contracts over the partition axis, so
    both matmul operands need the hidden dim on partitions); round x and W[0].T
    to the fp32r format (fp32 with 11 explicit mantissa bits, RNE) so the
    TensorE can run fp32r matmuls at 1 column/cycle (4x faster than fp32).
  - Device (per core): keep W[0].T resident in SBUF as fp32r [128, 8, 1024];
    stream 512-token chunks of x.T; 8 accumulating fp32r matmuls per
    (128-token, 512-feature) PSUM tile; bias-add on VectorE during PSUM->SBUF
    copyback; DMA out in natural token-major layout.
"""

import sys

if "/opt/trn_rl_repo" not in sys.path:
    sys.path.insert(0, "/opt/trn_rl_repo")

from contextlib import ExitStack

import numpy as np

import concourse.bass as bass
import concourse.tile as tile
from concourse import bacc, mybir
from concourse.bass_utils import run_bass_kernel_spmd

dt = mybir.dt

BATCH = 65536
HIDDEN = 1024
NCORES = 8
SHARD = BATCH // NCORES  # 8192 tokens per core
KD = HIDDEN // 128  # 8 hidden-dim blocks of 128
CHUNK = 1024  # tokens per streamed x chunk (4KB contiguous DMA runs)
NCHUNKS = SHARD // CHUNK  # 8
SM = CHUNK // 128  # 8 psum sub-tiles of 128 tokens per chunk


def round_fp32r(a: np.ndarray) -> np.ndarray:
    """Round fp32 to fp32r: 11 explicit mantissa bits, round-to-nearest-even."""
    bits = a.view(np.uint32).astype(np.uint64)
    lsb = (bits >> 12) & 1
    rounded = (bits + 0x7FF + lsb) & ~np.uint64(0xFFF)
    return rounded.astype(np.uint32).view(np.float32)


def build_program(loop_reps: int = 0, bench_mode: bool = False):
    """Build the per-core Bass program. loop_reps>0 wraps the main loop in a
    hardware For_i that repeats the whole computation (for benchmarking).

    bench_mode=True keeps x and out in Internal DRAM (no host transfer) so
    wall-clock timing of repeated runs is dominated by device execution; a tiny
    external output preserves a data dependency on the computation."""
    nc = bacc.Bacc("TRN2", debug=False, enable_asserts=True, num_devices=NCORES)
    io_kind = "Internal" if bench_mode else None
    xT_d = nc.dram_tensor(
        "xT", [HIDDEN, SHARD], dt.float32r, kind=io_kind or "ExternalInput"
    ).ap()
    w_d = nc.dram_tensor("w0t", [HIDDEN, HIDDEN], dt.float32r, kind="ExternalInput").ap()
    b_d = nc.dram_tensor("b0", [1, HIDDEN], dt.float32, kind="ExternalInput").ap()
    out_d = nc.dram_tensor(
        "out", [SHARD, HIDDEN], dt.float32, kind=io_kind or "ExternalOutput"
    ).ap()
    done_d = (
        nc.dram_tensor("done", [1, 16], dt.float32, kind="ExternalOutput").ap()
        if bench_mode
        else None
    )

    xT_v = xT_d.rearrange("(kd p) n -> p kd n", p=128)  # [128, 8, 8192]
    w_v = w_d.rearrange("(kd p) o -> p kd o", p=128)  # [128, 8, 1024]

    with tile.TileContext(nc) as tc:
        with ExitStack() as ctx:
            singles = ctx.enter_context(tc.tile_pool(name="singles", bufs=1))
            xpool = ctx.enter_context(tc.tile_pool(name="xpool", bufs=4))
            opool = ctx.enter_context(tc.tile_pool(name="opool", bufs=4))
            pspool = ctx.enter_context(tc.tile_pool(name="pspool", bufs=8, space="PSUM"))

            # Resident W[0].T in fp32r (one tile per 128-wide hidden block so
            # matmuls depend only on the slice they read) and broadcast bias.
            wts = []
            for kd in range(KD):
                wk = singles.tile([128, HIDDEN], dt.float32r, name=f"wt{kd}")
                nc.sync.dma_start(wk, w_v[:, kd, :])
                wts.append(wk)
            bias = singles.tile([128, HIDDEN], dt.float32, name="bias")
            nc.gpsimd.dma_start(
                bias, bass.AP(b_d.tensor, 0, [[0, 128], [1, HIDDEN]])
            )

            def chunk_body(ch: int):
                # one DMA + one tile per 128-wide hidden block: kd-block k's
                # matmuls unblock as soon as its slice lands
                xks = []
                for kd in range(KD):
                    xk = xpool.tile([128, CHUNK], dt.float32r, name=f"xk{kd}", tag=f"xk{kd}")
                    nc.sync.dma_start(xk, xT_v[:, kd, ch * CHUNK : (ch + 1) * CHUNK])
                    xks.append(xk)
                for sm in range(SM):
                    tok = ch * CHUNK + sm * 128
                    osb = opool.tile([128, HIDDEN], dt.float32, name="osb", tag="osb")
                    ps0 = pspool.tile([128, 512], dt.float32, name="ps0", tag="ps")
                    ps1 = pspool.tile([128, 512], dt.float32, name="ps1", tag="ps")
                    for kd in range(KD):
                        lhsT = xks[kd][:, sm * 128 : (sm + 1) * 128]
                        nc.tensor.matmul(
                            ps0, lhsT, wts[kd][:, 0:512],
                            start=(kd == 0), stop=(kd == KD - 1),
                        )
                        nc.tensor.matmul(
                            ps1, lhsT, wts[kd][:, 512:1024],
                            start=(kd == 0), stop=(kd == KD - 1),
                        )
                    nc.vector.tensor_add(osb[:, 0:512], ps0, bias[:, 0:512])
                    nc.vector.tensor_add(osb[:, 512:1024], ps1, bias[:, 512:1024])
                    nc.sync.dma_start(out_d[tok : tok + 128, :], osb)

            if bench_mode:
                # fp32r tiles may contain arbitrary bits in bench mode (x is
                # uninitialized Internal DRAM); zero the x region once so the
                # PE never chews on NaN/Inf patterns.
                zro = singles.tile([128, KD, 256], dt.float32r, name="zro")
                nc.vector.memset(zro.bitcast(dt.float32), 0.0)
                for zc in range(SHARD // 256):
                    nc.sync.dma_start(xT_v[:, :, zc * 256 : (zc + 1) * 256], zro)

            if loop_reps > 0:
                with tc.For_i(0, loop_reps, 1):
                    for ch in range(NCHUNKS):
                        chunk_body(ch)
            else:
                for ch in range(NCHUNKS):
                    chunk_body(ch)

            if done_d is not None:
                dsb = singles.tile([1, 16], dt.float32, name="dsb")
                nc.vector.tensor_copy(dsb, bias[0:1, 0:16])
                nc.sync.dma_start(done_d, dsb)

    nc.compile()
    return nc


_nc_cache: dict[tuple, object] = {}


def _get_nc(loop_reps: int = 0, bench_mode: bool = False):
    key = (loop_reps, bench_mode)
    if key not in _nc_cache:
        _nc_cache[key] = build_program(loop_reps, bench_mode)
    return _nc_cache[key]


def prepare_in_maps(x: np.ndarray, W: np.ndarray, b: np.ndarray):
    w0t_r = round_fp32r(np.ascontiguousarray(W[0].T))
    b0 = np.ascontiguousarray(b[0].reshape(1, HIDDEN)).astype(np.float32)
    in_maps = []
    for c in range(NCORES):
        x_c = x[c * SHARD : (c + 1) * SHARD]
        xT_c = round_fp32r(np.ascontiguousarray(x_c.T))
        in_maps.append({"xT": xT_c, "w0t": w0t_r, "b0": b0})
    return in_maps


def kernel(x, routing_vectors, W, b):
    x = np.asarray(x, dtype=np.float32)
    W = np.asarray(W, dtype=np.float32)
    b = np.asarray(b, dtype=np.float32)
    nc = _get_nc(0)
    in_maps = prepare_in_maps(x, W, b)
    res = run_bass_kernel_spmd(nc, in_maps, core_ids=list(range(NCORES)))
    return np.concatenate([res.results[c]["out"] for c in range(NCORES)], axis=0)


# revision 14
# speedup vs baseline: 1.0342x; 1.0342x over previous
"""Trainium2 Bass kernel for nn_MixtureOfAdaptors (moe_routing).

The reference routing collapses to expert 0 with weight 1.0, so the module is
exactly: out = x @ W[0].T + b[0], with x [65536, 1024] fp32.

Strategy (8 NeuronCores, data-parallel over tokens):
  - Host: shard x by token into 8 x [8192, 1024]; transpose each shard to
    feature-major [1024, 8192] (the PE contracts over the partition axis, so
    both matmul operands need the hidden dim on partitions); round x and W[0].T
    to the fp32r format (fp32 with 11 explicit mantissa bits, RNE) so the
    TensorE can run fp32r matmuls at 1 column/cycle (4x faster than fp32).
  - Device (per core): keep W[0].T resident in SBUF as fp32r (8 tiles of
    [128, 1024], one per 128-wide hidden block); stream 1024-token chunks of
    x.T as 8 per-block DMAs (4KB contiguous runs, fine-grained DMA->matmul
    dependencies); 8 accumulating fp32r matmuls per (128-token, 512-feature)
    PSUM tile, all 8 PSUM banks in flight; bias-add on VectorE during
    PSUM->SBUF copyback; DMA out in natural token-major layout.

    Measured steady-state: ~255-260us per core (PE-bound; 1024 matmul
    instructions x ~250ns; DMA ~237us overlapped).
"""

import sys

if "/opt/trn_rl_repo" not in sys.path:
    sys.path.insert(0, "/opt/trn_rl_repo")

from contextlib import ExitStack

import numpy as np

import concourse.bass as bass
import concourse.tile as tile
from concourse import bacc, mybir
from concourse.bass_utils import run_bass_kernel_spmd

dt = mybir.dt

BATCH = 65536
HIDDEN = 1024
NCORES = 8
SHARD = BATCH // NCORES  # 8192 tokens per core
KD = HIDDEN // 128  # 8 hidden-dim blocks of 128
CHUNK = 1024  # tokens per streamed x chunk (4KB contiguous DMA runs)
NCHUNKS = SHARD // CHUNK
SM = CHUNK // 128


def round_fp32r(a: np.ndarray) -> np.ndarray:
    """Round fp32 to fp32r: 11 explicit mantissa bits, round-to-nearest-even."""
    bits = a.view(np.uint32).astype(np.uint64)
    lsb = (bits >> 12) & 1
    rounded = (bits + 0x7FF + lsb) & ~np.uint64(0xFFF)
    return rounded.astype(np.uint32).view(np.float32)


def build_program(loop_reps: int = 0, bench_mode: bool = False):
    """Build the per-core Bass program. loop_reps>0 wraps the main loop in a
    hardware For_i that repeats the whole computation (for benchmarking).

    bench_mode=True keeps x and out in Internal DRAM (no host transfer) so
    wall-clock timing of repeated runs is dominated by device execution; a tiny
    external output preserves a data dependency on the computation."""
    nc = bacc.Bacc("TRN2", debug=False, enable_asserts=True, num_devices=NCORES)
    io_kind = "Internal" if bench_mode else None
    xT_d = nc.dram_tensor(
        "xT", [HIDDEN, SHARD], dt.float32r, kind=io_kind or "ExternalInput"
    ).ap()
    w_d = nc.dram_tensor("w0t", [HIDDEN, HIDDEN], dt.float32r, kind="ExternalInput").ap()
    b_d = nc.dram_tensor("b0", [1, HIDDEN], dt.float32, kind="ExternalInput").ap()
    out_d = nc.dram_tensor(
        "out", [SHARD, HIDDEN], dt.float32, kind=io_kind or "ExternalOutput"
    ).ap()
    done_d = (
        nc.dram_tensor("done", [1, 16], dt.float32, kind="ExternalOutput").ap()
        if bench_mode
        else None
    )

    xT_v = xT_d.rearrange("(kd p) n -> p kd n", p=128)  # [128, 8, 8192]
    w_v = w_d.rearrange("(kd p) o -> p kd o", p=128)  # [128, 8, 1024]

    with tile.TileContext(nc) as tc:
        with ExitStack() as ctx:
            singles = ctx.enter_context(tc.tile_pool(name="singles", bufs=1))
            xpool = ctx.enter_context(tc.tile_pool(name="xpool", bufs=4))
            opool = ctx.enter_context(tc.tile_pool(name="opool", bufs=4))
            pspool = ctx.enter_context(tc.tile_pool(name="pspool", bufs=8, space="PSUM"))

            # Resident W[0].T in fp32r (one tile per 128-wide hidden block so
            # matmuls depend only on the slice they read) and broadcast bias.
            wts = []
            for kd in range(KD):
                wk = singles.tile([128, HIDDEN], dt.float32r, name=f"wt{kd}")
                nc.sync.dma_start(wk, w_v[:, kd, :])
                wts.append(wk)
            bias = singles.tile([128, HIDDEN], dt.float32, name="bias")
            nc.gpsimd.dma_start(
                bias, bass.AP(b_d.tensor, 0, [[0, 128], [1, HIDDEN]])
            )

            def chunk_body(ch: int):
                # one DMA + one tile per 128-wide hidden block: kd-block k's
                # matmuls unblock as soon as its slice lands
                xks = []
                for kd in range(KD):
                    xk = xpool.tile([128, CHUNK], dt.float32r, name=f"xk{kd}", tag=f"xk{kd}")
                    nc.sync.dma_start(xk, xT_v[:, kd, ch * CHUNK : (ch + 1) * CHUNK])
                    xks.append(xk)
                for sm in range(SM):
                    tok = ch * CHUNK + sm * 128
                    osb = opool.tile([128, HIDDEN], dt.float32, name="osb", tag="osb")
                    ps0 = pspool.tile([128, 512], dt.float32, name="ps0", tag="ps")
                    ps1 = pspool.tile([128, 512], dt.float32, name="ps1", tag="ps")
                    for kd in range(KD):
                        lhsT = xks[kd][:, sm * 128 : (sm + 1) * 128]
                        nc.tensor.matmul(
                            ps0, lhsT, wts[kd][:, 0:512],
                            start=(kd == 0), stop=(kd == KD - 1),
                        )
                        nc.tensor.matmul(
                            ps1, lhsT, wts[kd][:, 512:1024],
                            start=(kd == 0), stop=(kd == KD - 1),
                        )
                    nc.vector.tensor_add(osb[:, 0:512], ps0, bias[:, 0:512])
                    nc.vector.tensor_add(osb[:, 512:1024], ps1, bias[:, 512:1024])
                    nc.sync.dma_start(out_d[tok : tok + 128, :], osb)

            if bench_mode:
                # fp32r tiles may contain arbitrary bits in bench mode (x is
                # uninitialized Internal DRAM); zero the x region once so the
                # PE never chews on NaN/Inf patterns.
                zro = singles.tile([128, KD, 256], dt.float32r, name="zro")
                nc.vector.memset(zro.bitcast(dt.float32), 0.0)
                for zc in range(SHARD // 256):
                    nc.sync.dma_start(xT_v[:, :, zc * 256 : (zc + 1) * 256], zro)

            if loop_reps > 0:
                with tc.For_i(0, loop_reps, 1):
                    for ch in range(NCHUNKS):
                        chunk_body(ch)
            else:
                for ch in range(NCHUNKS):
                    chunk_body(ch)

            if done_d is not None:
                dsb = singles.tile([1, 16], dt.float32, name="dsb")
                nc.vector.tensor_copy(dsb, bias[0:1, 0:16])
                nc.sync.dma_start(done_d, dsb)

    nc.compile()
    return nc


_nc_cache: dict[tuple, object] = {}


def _get_nc(loop_reps: int = 0, bench_mode: bool = False):
    key = (loop_reps, bench_mode)
    if key not in _nc_cache:
        _nc_cache[key] = build_program(loop_reps, bench_mode)
    return _nc_cache[key]


def prepare_in_maps(x: np.ndarray, W: np.ndarray, b: np.ndarray):
    w0t_r = round_fp32r(np.ascontiguousarray(W[0].T))
    b0 = np.ascontiguousarray(b[0].reshape(1, HIDDEN)).astype(np.float32)
    in_maps = []
    for c in range(NCORES):
        x_c = x[c * SHARD : (c + 1) * SHARD]
        xT_c = round_fp32r(np.ascontiguousarray(x_c.T))
        in_maps.append({"xT": xT_c, "w0t": w0t_r, "b0": b0})
    return in_maps


def kernel(x, routing_vectors, W, b):
    x = np.asarray(x, dtype=np.float32)
    W = np.asarray(W, dtype=np.float32)
    b = np.asarray(b, dtype=np.float32)
    nc = _get_nc(0)
    in_maps = prepare_in_maps(x, W, b)
    res = run_bass_kernel_spmd(nc, in_maps, core_ids=list(range(NCORES)))
    return np.concatenate([res.results[c]["out"] for c in range(NCORES)], axis=0)
